# revision 2
# baseline (speedup 1.0000x reference)
"""ClipNet top-K kernel v2 for 8 Trainium2 NeuronCores (pure data-parallel).

Math per batch row i (global i in 0..127):
  img   = normalize(input_images[i] @ W_img)            # [512]
  txt   = normalize(input_texts[i]  @ W_txt)            # [512]
  E     = other_texts[i] @ W_txt                        # [2048, 512]
  logit_oth = exp(ls) * (E @ img) / ||E||_row           # [2048]
  logit_in  = exp(ls) * (img . txt)
  out[i] = top127(logit_oth) sorted desc, with logit_in inserted at pos i

v3 changes vs baseline:
  - one batched DMA per row-slab (othT [512, 2048] -> [128, 4, 2048])
  - numerator via 1-col matmuls that reuse the E-matmul's stationary
    weights (PE cost ~0 vs 46us for the diagonal-masked variant)
  - nsq and num collected in column layout [128, (b, nch)]; rs computed
    in column layout (2 activations instead of per-row epilogue)
  - logits to row-major via PE transpose + batched gather DMAs
    (replaces 16 x 3.2us scatter DMAs)
  - exact hierarchical top-k: quarters of each row sorted to top-128 on
    [64, 512] (16 max8/match_replace rounds at 512 free-elems), then a
    merge pass over the 4 sorted lists on [16, 512]; 2x cheaper than
    16 rounds over [*, 2048]
"""

import os
import sys

import numpy as np

sys.path.insert(0, "/opt/trn_rl_repo")

import concourse.bacc as bacc
import concourse.tile as tile
from concourse import mybir
from concourse.masks import make_identity

F32 = mybir.dt.float32
U8 = mybir.dt.uint8
BF16 = mybir.dt.bfloat16

import ml_dtypes

MM_DT = BF16
NP_MM_DT = ml_dtypes.bfloat16

TOPK_DT = BF16 if os.environ.get("CLIP_TOPK_DT", "bf16") == "bf16" else F32

B = 128
N = 2048
F_IMG = 1024
F_TXT = 512
D = 512
K = 127          # topK = B - 1
NCORES = 8
BLOC = B // NCORES   # 16 rows per core
NEG = -1e30

KC = D // 128        # 4 contraction chunks of 128
NCH = N // 128       # 16 row-chunks of 128
NGROUP = 2
GR = BLOC // NGROUP  # 8 rows per epilogue group


def declare_params(nc):
    p = {}
    p["imgT"] = nc.declare_dram_parameter("imgT", [F_IMG, BLOC], MM_DT, isOutput=False)
    p["txtT"] = nc.declare_dram_parameter("txtT", [F_TXT, BLOC], MM_DT, isOutput=False)
    p["othT"] = nc.declare_dram_parameter("othT", [BLOC, F_TXT, N], MM_DT, isOutput=False)
    p["w_img"] = nc.declare_dram_parameter("w_img", [F_IMG, D], MM_DT, isOutput=False)
    p["w_txt"] = nc.declare_dram_parameter("w_txt", [F_TXT, D], MM_DT, isOutput=False)
    p["w_txtT"] = nc.declare_dram_parameter("w_txtT", [D, F_TXT], MM_DT, isOutput=False)
    p["m_lt"] = nc.declare_dram_parameter("m_lt", [BLOC, K + 1], U8, isOutput=False)
    p["m_eq"] = nc.declare_dram_parameter("m_eq", [BLOC, K + 1], U8, isOutput=False)
    p["ls"] = nc.declare_dram_parameter("ls", [1, 1], F32, isOutput=False)
    p["out"] = nc.declare_dram_parameter("out", [BLOC, K + 1], F32, isOutput=True)
    return p


def _build_kernel(tc, p, rep=""):
    nc = tc.nc
    out_dram = p["out"]

    Act = mybir.ActivationFunctionType

    with (
        tc.tile_pool(name=f"{rep}weights", bufs=1) as wpool,
        tc.tile_pool(name=f"{rep}small", bufs=1) as small,
        tc.tile_pool(name=f"{rep}xt", bufs=2) as xt_pool,
        tc.tile_pool(name=f"{rep}ps_e", bufs=2, space="PSUM") as ps_e,
        tc.tile_pool(name=f"{rep}ps_num", bufs=2, space="PSUM") as ps_num,
        tc.tile_pool(name=f"{rep}ps_tp", bufs=1, space="PSUM") as ps_tp,
    ):
        # ---------------- prologue: weights + embeddings ----------------
        prologue_psum = tc.tile_pool(name=f"{rep}ps_misc", bufs=1, space="PSUM")
        ps_misc = prologue_psum.__enter__()
        w_img_sb = wpool.tile([128, F_IMG // 128, D], MM_DT)
        nc.sync.dma_start(w_img_sb, p["w_img"][:].rearrange("(k p) d -> p k d", p=128))
        w_txt_sb = wpool.tile([128, KC, D], MM_DT)
        nc.sync.dma_start(w_txt_sb, p["w_txt"][:].rearrange("(k p) d -> p k d", p=128))
        w_txtT_sb = wpool.tile([128, KC, F_TXT], MM_DT)
        nc.sync.dma_start(w_txtT_sb, p["w_txtT"][:].rearrange("(k p) d -> p k d", p=128))

        imgT_sb = small.tile([128, F_IMG // 128, BLOC], MM_DT)
        nc.sync.dma_start(imgT_sb, p["imgT"][:].rearrange("(k p) m -> p k m", p=128))
        txtT_sb = small.tile([128, KC, BLOC], MM_DT)
        nc.sync.dma_start(txtT_sb, p["txtT"][:].rearrange("(k p) m -> p k m", p=128))

        m_lt_sb = small.tile([BLOC, K + 1], U8)
        nc.sync.dma_start(m_lt_sb, p["m_lt"][:])
        m_eq_sb = small.tile([BLOC, K + 1], U8)
        nc.sync.dma_start(m_eq_sb, p["m_eq"][:])

        identity = small.tile([128, 128], F32)
        make_identity(nc, identity)

        # img = imgT.T @ W_img   -> [16, 512] (accumulate 8 k-chunks)
        img_ps = ps_misc.tile([BLOC, D], F32, tag="misc")
        nkc_img = F_IMG // 128
        for k in range(nkc_img):
            nc.tensor.matmul(
                img_ps,
                lhsT=imgT_sb[:, k, :],
                rhs=w_img_sb[:, k, :],
                start=(k == 0),
                stop=(k == nkc_img - 1),
            )
        txt_ps = ps_misc.tile([BLOC, D], F32, tag="misc")
        for k in range(KC):
            nc.tensor.matmul(
                txt_ps,
                lhsT=txtT_sb[:, k, :],
                rhs=w_txt_sb[:, k, :],
                start=(k == 0),
                stop=(k == KC - 1),
            )

        # normalize rows of img / txt (copy PSUM->SBUF first: DVE reads
        # at most one PSUM operand)
        img_sb = small.tile([BLOC, D], F32)
        nc.vector.tensor_copy(img_sb, img_ps)
        sq_scr = small.tile([BLOC, D], F32)
        img_nsq = small.tile([BLOC, 1], F32)
        nc.scalar.activation(sq_scr, img_sb, Act.Square, accum_out=img_nsq)
        img_rn = small.tile([BLOC, 1], F32)
        nc.scalar.activation(img_rn, img_nsq, Act.Ln)
        nc.scalar.activation(img_rn, img_rn, Act.Exp, scale=-0.5)
        img_n = small.tile([BLOC, D], F32)
        nc.vector.tensor_scalar_mul(img_n, img_sb, scalar1=img_rn)

        txt_sb = small.tile([BLOC, D], F32)
        nc.vector.tensor_copy(txt_sb, txt_ps)
        sq_scr2 = small.tile([BLOC, D], F32)
        txt_nsq = small.tile([BLOC, 1], F32)
        nc.scalar.activation(sq_scr2, txt_sb, Act.Square, accum_out=txt_nsq)
        txt_rn = small.tile([BLOC, 1], F32)
        nc.scalar.activation(txt_rn, txt_nsq, Act.Ln)
        nc.scalar.activation(txt_rn, txt_rn, Act.Exp, scale=-0.5)
        txt_n = small.tile([BLOC, D], F32)
        nc.vector.tensor_scalar_mul(txt_n, txt_sb, scalar1=txt_rn)

        # logit_in (unscaled) = rowsum(img_n * txt_n)
        prod_it = small.tile([BLOC, D], F32)
        nc.vector.tensor_mul(prod_it, img_n, txt_n)
        sq_scr3 = small.tile([BLOC, D], F32)
        li_raw = small.tile([BLOC, 1], F32)
        nc.scalar.activation(sq_scr3, prod_it, Act.Copy, accum_out=li_raw)

        # broadcast ls to [128,1] via DMA (src partition stride 0)
        import concourse.bass as bass_mod
        ls_ap = p["ls"][:]
        ls_b128 = bass_mod.AP(
            tensor=ls_ap.tensor, offset=ls_ap.offset, ap=[[0, 128], [1, 1]]
        )
        ls128 = small.tile([128, 1], F32)
        nc.sync.dma_start(ls128, ls_b128)
        sc16 = small.tile([BLOC, 1], F32)
        nc.scalar.activation(sc16, ls128[0:BLOC, :], Act.Exp)

        li = small.tile([BLOC, 1], F32)
        nc.vector.tensor_mul(li, li_raw, sc16)

        # img_n^T  [512, 16] via PE transposes of [16,128] slices
        imgnT_sb = small.tile([128, KC, BLOC], MM_DT)
        for c in range(KC):
            tp_ps = ps_misc.tile([128, BLOC], F32, tag="misc")
            nc.tensor.transpose(tp_ps, img_n[:, 128 * c:128 * (c + 1)],
                                identity[:BLOC, :BLOC])
            nc.vector.tensor_copy(imgnT_sb[:, c, :], tp_ps)

        # V[k, b] = sum_j W_txt[k, j] img_n[b, j]  -> v_sb [128, KC, 16]
        v_sb = small.tile([128, KC, BLOC], MM_DT)
        for kcc in range(KC):
            v_ps = ps_misc.tile([128, BLOC], F32, tag="misc")
            for j in range(KC):
                nc.tensor.matmul(
                    v_ps,
                    lhsT=w_txtT_sb[:, j, 128 * kcc:128 * (kcc + 1)],
                    rhs=imgnT_sb[:, j, :],
                    start=(j == 0),
                    stop=(j == KC - 1),
                )
            nc.vector.tensor_copy(v_sb[:, kcc, :], v_ps)
        prologue_psum.__exit__(None, None, None)

        # ---------------- streaming loop over the 16 batch rows ----------------
        nsq_cols = small.tile([128, BLOC * NCH], F32)   # col c = b*16 + nch
        num_cols = small.tile([128, BLOC * NCH], F32)
        rs_cols = small.tile([128, BLOC * NCH], F32)
        log_cols = small.tile([128, BLOC * NCH], F32)
        logT = [small.tile([128, 128], F32, name=f"{rep}logT{t}") for t in range(2)]
        # quarters layout: partition q = b*4 + nq holds row b's quarter nq
        rowlay4 = small.tile([4 * BLOC, N // 4], F32)
        st1 = small.tile([4 * BLOC, 128], F32)          # per-quarter sorted 128
        merge_in = small.tile([BLOC, 512], F32)
        topk_sb = small.tile([BLOC, 128], F32)

        # hoist xt_0 DMA ahead of the prologue compute
        xts = {}
        xts[0] = xt_pool.tile([128, KC, N], MM_DT, tag="xt", name=f"{rep}xt_0")
        nc.sync.dma_start(xts[0], p["othT"][0].rearrange("(k p) n -> p k n", p=128))

        for b in range(BLOC):
            xt = xts.pop(b)
            if b + 1 < BLOC:
                xts[b + 1] = xt_pool.tile([128, KC, N], MM_DT, tag="xt",
                                          name=f"{rep}xt_{b + 1}")
                nc.sync.dma_start(
                    xts[b + 1],
                    p["othT"][b + 1].rearrange("(k p) n -> p k n", p=128),
                )

            nmp = ps_num.tile([128, BLOC], F32, tag="nm", name=f"{rep}nm_{b}")
            for nch in range(NCH):
                e_ps = ps_e.tile([128, D], F32, tag="e")
                for kcc in range(KC):
                    lhsT = xt[:, kcc, 128 * nch:128 * (nch + 1)]
                    nc.tensor.matmul(
                        e_ps, lhsT=lhsT, rhs=w_txt_sb[:, kcc, :],
                        start=(kcc == 0), stop=(kcc == KC - 1),
                    )
                    # numerator: same stationary weights, 1-col moving
                    nc.tensor.matmul(
                        nmp[:, nch:nch + 1], lhsT=lhsT,
                        rhs=v_sb[:, kcc, b:b + 1],
                        start=(kcc == 0), stop=(kcc == KC - 1),
                    )
                nc.scalar.activation(
                    e_ps, e_ps, Act.Square,
                    accum_out=nsq_cols[:, BLOC * b + nch: BLOC * b + nch + 1],
                )
            nc.vector.tensor_copy(num_cols[:, BLOC * b:BLOC * (b + 1)], nmp)

        # ---------------- epilogue ----------------
        # rs = exp(ls - 0.5 ln nsq), logits = num * rs, all in column layout
        nc.scalar.activation(rs_cols, nsq_cols, Act.Ln)
        nc.scalar.activation(rs_cols, rs_cols, Act.Exp, scale=-0.5, bias=ls128)
        nc.vector.tensor_mul(log_cols, num_cols, rs_cols)

        # transpose to [c = b*16+nch, p] and scatter quarters to partitions:
        # rowlay4[c//4, (c%4)*128 + p] = logT[c, p]  (c//4 = b*4 + nch//4)
        for t in range(2):
            tp = ps_tp.tile([128, 128], F32, tag="tp")
            nc.tensor.transpose(tp, log_cols[:, 128 * t:128 * (t + 1)], identity)
            nc.vector.tensor_copy(logT[t], tp)
            nc.sync.dma_start(
                rowlay4[32 * t:32 * (t + 1), :].rearrange(
                    "q (c p) -> q c p", c=4),
                logT[t][:],
            )

        # stage 1: sorted top-128 of each quarter  [64, 512] -> [64, 128]
        for i in range(16):
            nc.vector.max(out=st1[:, 8 * i:8 * i + 8], in_=rowlay4)
            nc.vector.match_replace(
                out=rowlay4, in_to_replace=st1[:, 8 * i:8 * i + 8],
                in_values=rowlay4, imm_value=NEG,
            )

        # concat the 4 sorted lists of each row: merge_in[b, nq*128+k]
        nc.sync.dma_start(
            merge_in[:].rearrange("b (nq k) -> b nq k", nq=4), st1[:]
        )

        # stage 2: top-128 of the 512 candidates per row
        for i in range(16):
            nc.vector.max(out=topk_sb[:, 8 * i:8 * i + 8], in_=merge_in)
            nc.vector.match_replace(
                out=merge_in, in_to_replace=topk_sb[:, 8 * i:8 * i + 8],
                in_values=merge_in, imm_value=NEG,
            )

        # insert logit_in at column i (global row index): masks from host
        shifted = small.tile([BLOC, K + 1], F32)
        nc.vector.tensor_copy(shifted[:, 1:K + 1], topk_sb[:, 0:K])
        nc.vector.tensor_copy(shifted[:, 0:1], topk_sb[:, 0:1])
        outt = small.tile([BLOC, K + 1], F32)
        nc.vector.select(outt, m_lt_sb, on_true=topk_sb, on_false=shifted)
        nc.vector.copy_predicated(outt, m_eq_sb, li.to_broadcast([BLOC, K + 1]))
        nc.sync.dma_start(out_dram[:], outt)

    return out_dram


def build_module(reps=1):
    nc = bacc.Bacc("TRN2", target_bir_lowering=False, debug=False, num_devices=NCORES)
    with tile.TileContext(nc) as tc:
        p = declare_params(nc)
        for r in range(reps):
            _build_kernel(tc, p, rep=f"r{r}_" if reps > 1 else "")
    nc.compile()
    return nc


def make_in_maps(input_images, input_texts, other_texts, W_img, W_txt, logit_scale):
    input_images = np.asarray(input_images, np.float32)
    input_texts = np.asarray(input_texts, np.float32)
    other_texts = np.asarray(other_texts, np.float32)
    W_img = np.ascontiguousarray(np.asarray(W_img, np.float32))
    W_txt = np.ascontiguousarray(np.asarray(W_txt, np.float32))
    W_txtT = np.ascontiguousarray(W_txt.T)
    ls = np.float32(np.asarray(logit_scale).reshape(-1)[0])

    cols = np.arange(K + 1)
    in_maps = []
    for c in range(NCORES):
        r = slice(BLOC * c, BLOC * (c + 1))
        gi = np.arange(BLOC * c, BLOC * (c + 1))[:, None]  # global row ids
        in_maps.append({
            "imgT": np.ascontiguousarray(input_images[r].T).astype(NP_MM_DT),
            "txtT": np.ascontiguousarray(input_texts[r].T).astype(NP_MM_DT),
            "othT": np.ascontiguousarray(other_texts[r].transpose(0, 2, 1)).astype(NP_MM_DT),
            "w_img": W_img.astype(NP_MM_DT),
            "w_txt": W_txt.astype(NP_MM_DT),
            "w_txtT": W_txtT.astype(NP_MM_DT),
            "m_lt": (cols[None, :] < gi).astype(np.uint8),
            "m_eq": (cols[None, :] == gi).astype(np.uint8),
            "ls": np.array([[ls]], np.float32),
        })
    return in_maps


_NC_CACHE = {}


def kernel(input_images, input_texts, other_texts, W_img, W_txt, logit_scale):
    from concourse.bass_utils import run_bass_kernel_spmd

    if "nc" not in _NC_CACHE:
        _NC_CACHE["nc"] = build_module()
    nc = _NC_CACHE["nc"]

    in_maps = make_in_maps(
        input_images, input_texts, other_texts, W_img, W_txt, logit_scale
    )
    res = run_bass_kernel_spmd(nc, in_maps, list(range(NCORES)))
    _NC_CACHE["last_result"] = res
    return np.concatenate([res.results[c]["out"] for c in range(NCORES)], axis=0)


# revision 3
# speedup vs baseline: 7.2043x; 7.2043x over previous
"""ClipNet top-K kernel for 8 Trainium2 NeuronCores (pure data-parallel).

Math per batch row i (global i in 0..127):
  img   = normalize(input_images[i] @ W_img)            # [512]
  txt   = normalize(input_texts[i]  @ W_txt)            # [512]
  E     = other_texts[i] @ W_txt                        # [2048, 512]
  logit_oth = exp(ls) * (E @ img) / ||E||_row           # [2048]
  logit_in  = exp(ls) * (img . txt)
  out[i] = top127(logit_oth) sorted desc, with logit_in inserted at pos i

v4 changes vs baseline (on top of the v3 epilogue/top-k restructuring):
  - E-matmuls in fp8 (e4m3) with DoubleRow perf mode (2 contraction
    chunks of 256); norms tolerate fp8 since the error averages over
    D=512 in ||E||^2
  - numerator keeps full precision via residual decomposition:
    x = fp8(x) + fp8(x - fp8(x)); num = x8.v + xr.v with v in bf16
    (mixed fp8 x bf16 matmuls, 1-col moving operand)
  - row-norm squares split across three engines: Act (Square+accum),
    and Pool (square) + DVE (free-axis reduce), interleaved per chunk

v3 changes vs baseline:
  - one batched DMA per row-slab (othT [512, 2048] -> [128, 4, 2048])
  - numerator via 1-col matmuls that reuse the E-matmul's stationary
    weights (PE cost ~0 vs 46us for the diagonal-masked variant)
  - nsq and num collected in column layout [128, (b, nch)]; rs computed
    in column layout (2 activations instead of per-row epilogue)
  - logits to row-major via PE transpose + batched gather DMAs
    (replaces 16 x 3.2us scatter DMAs)
  - exact hierarchical top-k: quarters of each row sorted to top-128 on
    [64, 512] (16 max8/match_replace rounds at 512 free-elems), then a
    merge pass over the 4 sorted lists on [16, 512]; 2x cheaper than
    16 rounds over [*, 2048]
"""

import os
import sys

import numpy as np

sys.path.insert(0, "/opt/trn_rl_repo")

import concourse.bacc as bacc
import concourse.tile as tile
from concourse import mybir
from concourse.masks import make_identity

F32 = mybir.dt.float32
U8 = mybir.dt.uint8
BF16 = mybir.dt.bfloat16
FP8 = mybir.dt.float8e4

import ml_dtypes

MM_DT = BF16
NP_MM_DT = ml_dtypes.bfloat16
NP_FP8 = ml_dtypes.float8_e4m3

TOPK_DT = BF16 if os.environ.get("CLIP_TOPK_DT", "bf16") == "bf16" else F32

B = 128
N = 2048
F_IMG = 1024
F_TXT = 512
D = 512
K = 127          # topK = B - 1
NCORES = 8
BLOC = B // NCORES   # 16 rows per core
NEG = -1e30

KC = D // 128        # 4 contraction chunks of 128
NCH = N // 128       # 16 row-chunks of 128
NGROUP = 2
GR = BLOC // NGROUP  # 8 rows per epilogue group


def declare_params(nc):
    p = {}
    p["imgT"] = nc.declare_dram_parameter("imgT", [F_IMG, BLOC], MM_DT, isOutput=False)
    p["txtT"] = nc.declare_dram_parameter("txtT", [F_TXT, BLOC], MM_DT, isOutput=False)
    p["othT8"] = nc.declare_dram_parameter("othT8", [BLOC, F_TXT, N], FP8, isOutput=False)
    p["othTr"] = nc.declare_dram_parameter("othTr", [BLOC, F_TXT, N], FP8, isOutput=False)
    p["w_img"] = nc.declare_dram_parameter("w_img", [F_IMG, D], MM_DT, isOutput=False)
    p["w_txt"] = nc.declare_dram_parameter("w_txt", [F_TXT, D], MM_DT, isOutput=False)
    p["w_txt8"] = nc.declare_dram_parameter("w_txt8", [F_TXT, D], FP8, isOutput=False)
    p["w_txtT"] = nc.declare_dram_parameter("w_txtT", [D, F_TXT], MM_DT, isOutput=False)
    p["m_lt"] = nc.declare_dram_parameter("m_lt", [BLOC, K + 1], U8, isOutput=False)
    p["m_eq"] = nc.declare_dram_parameter("m_eq", [BLOC, K + 1], U8, isOutput=False)
    p["ls"] = nc.declare_dram_parameter("ls", [1, 1], F32, isOutput=False)
    p["out"] = nc.declare_dram_parameter("out", [BLOC, K + 1], F32, isOutput=True)
    return p


def _build_kernel(tc, p, rep=""):
    nc = tc.nc
    out_dram = p["out"]

    Act = mybir.ActivationFunctionType

    with (
        tc.tile_pool(name=f"{rep}weights", bufs=1) as wpool,
        tc.tile_pool(name=f"{rep}small", bufs=1) as small,
        tc.tile_pool(name=f"{rep}xt", bufs=2) as xt_pool,
        tc.tile_pool(name=f"{rep}ps_e", bufs=4, space="PSUM") as ps_e,
        tc.tile_pool(name=f"{rep}ps_num", bufs=2, space="PSUM") as ps_num,
        tc.tile_pool(name=f"{rep}ps_tp", bufs=1, space="PSUM") as ps_tp,
    ):
        # ---------------- prologue: weights + embeddings ----------------
        prologue_psum = tc.tile_pool(name=f"{rep}ps_misc", bufs=1, space="PSUM")
        ps_misc = prologue_psum.__enter__()
        w_img_sb = wpool.tile([128, F_IMG // 128, D], MM_DT)
        nc.sync.dma_start(w_img_sb, p["w_img"][:].rearrange("(k p) d -> p k d", p=128))
        w_txt_sb = wpool.tile([128, KC, D], MM_DT)
        nc.sync.dma_start(w_txt_sb, p["w_txt"][:].rearrange("(k p) d -> p k d", p=128))
        w_txtT_sb = wpool.tile([128, KC, F_TXT], MM_DT)
        nc.sync.dma_start(w_txtT_sb, p["w_txtT"][:].rearrange("(k p) d -> p k d", p=128))
        w8_sb = wpool.tile([128, KC, D], FP8)
        nc.sync.dma_start(w8_sb, p["w_txt8"][:].rearrange("(k p) d -> p k d", p=128))

        imgT_sb = small.tile([128, F_IMG // 128, BLOC], MM_DT)
        nc.sync.dma_start(imgT_sb, p["imgT"][:].rearrange("(k p) m -> p k m", p=128))
        txtT_sb = small.tile([128, KC, BLOC], MM_DT)
        nc.sync.dma_start(txtT_sb, p["txtT"][:].rearrange("(k p) m -> p k m", p=128))

        m_lt_sb = small.tile([BLOC, K + 1], U8)
        nc.sync.dma_start(m_lt_sb, p["m_lt"][:])
        m_eq_sb = small.tile([BLOC, K + 1], U8)
        nc.sync.dma_start(m_eq_sb, p["m_eq"][:])

        identity = small.tile([128, 128], F32)
        make_identity(nc, identity)

        # img = imgT.T @ W_img   -> [16, 512] (accumulate 8 k-chunks)
        img_ps = ps_misc.tile([BLOC, D], F32, tag="misc")
        nkc_img = F_IMG // 128
        for k in range(nkc_img):
            nc.tensor.matmul(
                img_ps,
                lhsT=imgT_sb[:, k, :],
                rhs=w_img_sb[:, k, :],
                start=(k == 0),
                stop=(k == nkc_img - 1),
            )
        txt_ps = ps_misc.tile([BLOC, D], F32, tag="misc")
        for k in range(KC):
            nc.tensor.matmul(
                txt_ps,
                lhsT=txtT_sb[:, k, :],
                rhs=w_txt_sb[:, k, :],
                start=(k == 0),
                stop=(k == KC - 1),
            )

        # normalize rows of img / txt (copy PSUM->SBUF first: DVE reads
        # at most one PSUM operand)
        img_sb = small.tile([BLOC, D], F32)
        nc.vector.tensor_copy(img_sb, img_ps)
        sq_scr = small.tile([BLOC, D], F32)
        img_nsq = small.tile([BLOC, 1], F32)
        nc.scalar.activation(sq_scr, img_sb, Act.Square, accum_out=img_nsq)
        img_rn = small.tile([BLOC, 1], F32)
        nc.scalar.activation(img_rn, img_nsq, Act.Ln)
        nc.scalar.activation(img_rn, img_rn, Act.Exp, scale=-0.5)
        img_n = small.tile([BLOC, D], F32)
        nc.vector.tensor_scalar_mul(img_n, img_sb, scalar1=img_rn)

        txt_sb = small.tile([BLOC, D], F32)
        nc.vector.tensor_copy(txt_sb, txt_ps)
        sq_scr2 = small.tile([BLOC, D], F32)
        txt_nsq = small.tile([BLOC, 1], F32)
        nc.scalar.activation(sq_scr2, txt_sb, Act.Square, accum_out=txt_nsq)
        txt_rn = small.tile([BLOC, 1], F32)
        nc.scalar.activation(txt_rn, txt_nsq, Act.Ln)
        nc.scalar.activation(txt_rn, txt_rn, Act.Exp, scale=-0.5)
        txt_n = small.tile([BLOC, D], F32)
        nc.vector.tensor_scalar_mul(txt_n, txt_sb, scalar1=txt_rn)

        # logit_in (unscaled) = rowsum(img_n * txt_n)
        prod_it = small.tile([BLOC, D], F32)
        nc.vector.tensor_mul(prod_it, img_n, txt_n)
        sq_scr3 = small.tile([BLOC, D], F32)
        li_raw = small.tile([BLOC, 1], F32)
        nc.scalar.activation(sq_scr3, prod_it, Act.Copy, accum_out=li_raw)

        # broadcast ls to [128,1] via DMA (src partition stride 0)
        import concourse.bass as bass_mod
        ls_ap = p["ls"][:]
        ls_b128 = bass_mod.AP(
            tensor=ls_ap.tensor, offset=ls_ap.offset, ap=[[0, 128], [1, 1]]
        )
        ls128 = small.tile([128, 1], F32)
        nc.sync.dma_start(ls128, ls_b128)
        sc16 = small.tile([BLOC, 1], F32)
        nc.scalar.activation(sc16, ls128[0:BLOC, :], Act.Exp)

        li = small.tile([BLOC, 1], F32)
        nc.vector.tensor_mul(li, li_raw, sc16)

        # img_n^T  [512, 16] via PE transposes of [16,128] slices
        imgnT_sb = small.tile([128, KC, BLOC], MM_DT)
        for c in range(KC):
            tp_ps = ps_misc.tile([128, BLOC], F32, tag="misc")
            nc.tensor.transpose(tp_ps, img_n[:, 128 * c:128 * (c + 1)],
                                identity[:BLOC, :BLOC])
            nc.vector.tensor_copy(imgnT_sb[:, c, :], tp_ps)

        # V[k, b] = sum_j W_txt[k, j] img_n[b, j]  -> v_sb [128, KC, 16]
        v_sb = small.tile([128, KC, BLOC], MM_DT)
        for kcc in range(KC):
            v_ps = ps_misc.tile([128, BLOC], F32, tag="misc")
            for j in range(KC):
                nc.tensor.matmul(
                    v_ps,
                    lhsT=w_txtT_sb[:, j, 128 * kcc:128 * (kcc + 1)],
                    rhs=imgnT_sb[:, j, :],
                    start=(j == 0),
                    stop=(j == KC - 1),
                )
            nc.vector.tensor_copy(v_sb[:, kcc, :], v_ps)
        prologue_psum.__exit__(None, None, None)

        # ---------------- streaming loop over the 16 batch rows ----------------
        nsq_cols = small.tile([128, BLOC * NCH], F32)   # col c = b*16 + nch
        num_cols = small.tile([128, BLOC * NCH], F32)
        rs_cols = small.tile([128, BLOC * NCH], F32)
        log_cols = small.tile([128, BLOC * NCH], F32)
        logT = [small.tile([128, 128], F32, name=f"{rep}logT{t}") for t in range(2)]
        # quarters layout: partition q = b*4 + nq holds row b's quarter nq
        rowlay4 = small.tile([4 * BLOC, N // 4], F32)
        st1 = small.tile([4 * BLOC, 128], F32)          # per-quarter sorted 128
        merge_in = small.tile([BLOC, 512], F32)
        topk_sb = small.tile([BLOC, 128], F32)

        scr_pool = tc.tile_pool(name=f"{rep}scr", bufs=4)
        scr = scr_pool.__enter__()

        def load_xt(b):
            x8 = xt_pool.tile([128, KC, N], FP8, tag="x8", name=f"{rep}x8_{b}")
            nc.sync.dma_start(x8, p["othT8"][b].rearrange("(k p) n -> p k n", p=128))
            xr = xt_pool.tile([128, KC, N], FP8, tag="xr", name=f"{rep}xr_{b}")
            nc.sync.dma_start(xr, p["othTr"][b].rearrange("(k p) n -> p k n", p=128))
            return (x8, xr)

        # hoist xt_0 DMA ahead of the prologue compute
        xts = {0: load_xt(0)}

        # squares: interleaved Act / (DVE copy + Pool square + DVE reduce).
        # GPSIMD cannot read PSUM (walrus birverifier rule), so the Pool
        # path goes through a bf16 SBUF copy of the E tile first.
        SQ_ACT = {0, 2, 3, 5, 6, 8, 9, 11, 12, 14}

        for b in range(BLOC):
            x8, xr = xts.pop(b)
            if b + 1 < BLOC:
                xts[b + 1] = load_xt(b + 1)

            nmp = ps_num.tile([128, BLOC], F32, tag="nm", name=f"{rep}nm_{b}")
            for nch in range(NCH):
                sl = slice(128 * nch, 128 * (nch + 1))
                e_ps = ps_e.tile([128, D], F32, tag="e")
                for kp in range(KC // 2):
                    nc.tensor.matmul(
                        e_ps, lhsT=x8[:, 2 * kp:2 * kp + 2, sl],
                        rhs=w8_sb[:, 2 * kp:2 * kp + 2, :],
                        start=(kp == 0), stop=(kp == KC // 2 - 1),
                        perf_mode=mybir.MatmulPerfMode.DoubleRow,
                    )
                # numerator: num = (x8 + xr) . v   (v stays bf16)
                for kcc in range(KC):
                    nc.tensor.matmul(
                        nmp[:, nch:nch + 1], lhsT=x8[:, kcc, sl],
                        rhs=v_sb[:, kcc, b:b + 1],
                        start=(kcc == 0), stop=False,
                    )
                for kcc in range(KC):
                    nc.tensor.matmul(
                        nmp[:, nch:nch + 1], lhsT=xr[:, kcc, sl],
                        rhs=v_sb[:, kcc, b:b + 1],
                        start=False, stop=(kcc == KC - 1),
                    )
                col = nsq_cols[:, BLOC * b + nch: BLOC * b + nch + 1]
                if nch in SQ_ACT:
                    nc.scalar.activation(e_ps, e_ps, Act.Square, accum_out=col)
                else:
                    e_sb = scr.tile([128, D], BF16, tag="esb")
                    nc.vector.tensor_copy(e_sb, e_ps)
                    s = scr.tile([128, D], BF16, tag="sp")
                    nc.gpsimd.tensor_mul(s, e_sb, e_sb)
                    nc.vector.tensor_reduce(out=col, in_=s,
                                            op=mybir.AluOpType.add,
                                            axis=mybir.AxisListType.X)
            nc.vector.tensor_copy(num_cols[:, BLOC * b:BLOC * (b + 1)], nmp)
        scr_pool.__exit__(None, None, None)

        # ---------------- epilogue ----------------
        # rs = exp(ls - 0.5 ln nsq), logits = num * rs, all in column layout
        nc.scalar.activation(rs_cols, nsq_cols, Act.Ln)
        nc.scalar.activation(rs_cols, rs_cols, Act.Exp, scale=-0.5, bias=ls128)
        nc.vector.tensor_mul(log_cols, num_cols, rs_cols)

        # transpose to [c = b*16+nch, p] and scatter quarters to partitions:
        # rowlay4[c//4, (c%4)*128 + p] = logT[c, p]  (c//4 = b*4 + nch//4)
        for t in range(2):
            tp = ps_tp.tile([128, 128], F32, tag="tp")
            nc.tensor.transpose(tp, log_cols[:, 128 * t:128 * (t + 1)], identity)
            nc.vector.tensor_copy(logT[t], tp)
            nc.sync.dma_start(
                rowlay4[32 * t:32 * (t + 1), :].rearrange(
                    "q (c p) -> q c p", c=4),
                logT[t][:],
            )

        # stage 1: sorted top-128 of each quarter  [64, 512] -> [64, 128]
        for i in range(16):
            nc.vector.max(out=st1[:, 8 * i:8 * i + 8], in_=rowlay4)
            nc.vector.match_replace(
                out=rowlay4, in_to_replace=st1[:, 8 * i:8 * i + 8],
                in_values=rowlay4, imm_value=NEG,
            )

        # concat the 4 sorted lists of each row: merge_in[b, nq*128+k]
        nc.sync.dma_start(
            merge_in[:].rearrange("b (nq k) -> b nq k", nq=4), st1[:]
        )

        # stage 2: top-128 of the 512 candidates per row
        for i in range(16):
            nc.vector.max(out=topk_sb[:, 8 * i:8 * i + 8], in_=merge_in)
            nc.vector.match_replace(
                out=merge_in, in_to_replace=topk_sb[:, 8 * i:8 * i + 8],
                in_values=merge_in, imm_value=NEG,
            )

        # insert logit_in at column i (global row index): masks from host
        shifted = small.tile([BLOC, K + 1], F32)
        nc.vector.tensor_copy(shifted[:, 1:K + 1], topk_sb[:, 0:K])
        nc.vector.tensor_copy(shifted[:, 0:1], topk_sb[:, 0:1])
        outt = small.tile([BLOC, K + 1], F32)
        nc.vector.select(outt, m_lt_sb, on_true=topk_sb, on_false=shifted)
        nc.vector.copy_predicated(outt, m_eq_sb, li.to_broadcast([BLOC, K + 1]))
        nc.sync.dma_start(out_dram[:], outt)

    return out_dram


def build_module(reps=1):
    nc = bacc.Bacc("TRN2", target_bir_lowering=False, debug=False, num_devices=NCORES)
    with tile.TileContext(nc) as tc:
        p = declare_params(nc)
        for r in range(reps):
            _build_kernel(tc, p, rep=f"r{r}_" if reps > 1 else "")
    nc.compile()
    return nc


def make_in_maps(input_images, input_texts, other_texts, W_img, W_txt, logit_scale):
    input_images = np.asarray(input_images, np.float32)
    input_texts = np.asarray(input_texts, np.float32)
    other_texts = np.asarray(other_texts, np.float32)
    W_img = np.ascontiguousarray(np.asarray(W_img, np.float32))
    W_txt = np.ascontiguousarray(np.asarray(W_txt, np.float32))
    W_txtT = np.ascontiguousarray(W_txt.T)
    ls = np.float32(np.asarray(logit_scale).reshape(-1)[0])

    cols = np.arange(K + 1)
    in_maps = []
    for c in range(NCORES):
        r = slice(BLOC * c, BLOC * (c + 1))
        gi = np.arange(BLOC * c, BLOC * (c + 1))[:, None]  # global row ids
        othT = np.ascontiguousarray(other_texts[r].transpose(0, 2, 1))
        oth8 = othT.astype(NP_FP8)
        othr = (othT - oth8.astype(np.float32)).astype(NP_FP8)
        in_maps.append({
            "imgT": np.ascontiguousarray(input_images[r].T).astype(NP_MM_DT),
            "txtT": np.ascontiguousarray(input_texts[r].T).astype(NP_MM_DT),
            "othT8": oth8,
            "othTr": othr,
            "w_img": W_img.astype(NP_MM_DT),
            "w_txt": W_txt.astype(NP_MM_DT),
            "w_txt8": W_txt.astype(NP_FP8),
            "w_txtT": W_txtT.astype(NP_MM_DT),
            "m_lt": (cols[None, :] < gi).astype(np.uint8),
            "m_eq": (cols[None, :] == gi).astype(np.uint8),
            "ls": np.array([[ls]], np.float32),
        })
    return in_maps


_NC_CACHE = {}


def kernel(input_images, input_texts, other_texts, W_img, W_txt, logit_scale):
    from concourse.bass_utils import run_bass_kernel_spmd

    if "nc" not in _NC_CACHE:
        _NC_CACHE["nc"] = build_module()
    nc = _NC_CACHE["nc"]

    in_maps = make_in_maps(
        input_images, input_texts, other_texts, W_img, W_txt, logit_scale
    )
    res = run_bass_kernel_spmd(nc, in_maps, list(range(NCORES)))
    _NC_CACHE["last_result"] = res
    return np.concatenate([res.results[c]["out"] for c in range(NCORES)], axis=0)


# revision 12
# speedup vs baseline: 7.7972x; 1.0823x over previous
"""ClipNet top-K kernel for 8 Trainium2 NeuronCores (pure data-parallel).

Math per batch row i (global i in 0..127):
  img   = normalize(input_images[i] @ W_img)            # [512]
  txt   = normalize(input_texts[i]  @ W_txt)            # [512]
  E     = other_texts[i] @ W_txt                        # [2048, 512]
  logit_oth = exp(ls) * (E @ img) / ||E||_row           # [2048]
  logit_in  = exp(ls) * (img . txt)
  out[i] = top127(logit_oth) sorted desc, with logit_in inserted at pos i

v4 changes vs baseline (on top of the v3 epilogue/top-k restructuring):
  - E-matmuls in fp8 (e4m3) with DoubleRow perf mode (2 contraction
    chunks of 256); norms tolerate fp8 since the error averages over
    D=512 in ||E||^2
  - numerator keeps full precision via residual decomposition:
    x = fp8(x) + fp8(x - fp8(x)); num = x8.v + xr.v with v in bf16
    (mixed fp8 x bf16 matmuls, 1-col moving operand)
  - row-norm squares split across three engines: Act (Square+accum),
    and Pool (square) + DVE (free-axis reduce), interleaved per chunk

v3 changes vs baseline:
  - one batched DMA per row-slab (othT [512, 2048] -> [128, 4, 2048])
  - numerator via 1-col matmuls that reuse the E-matmul's stationary
    weights (PE cost ~0 vs 46us for the diagonal-masked variant)
  - nsq and num collected in column layout [128, (b, nch)]; rs computed
    in column layout (2 activations instead of per-row epilogue)
  - logits to row-major via PE transpose + batched gather DMAs
    (replaces 16 x 3.2us scatter DMAs)
  - exact hierarchical top-k: quarters of each row sorted to top-128 on
    [64, 512] (16 max8/match_replace rounds at 512 free-elems), then a
    merge pass over the 4 sorted lists on [16, 512]; 2x cheaper than
    16 rounds over [*, 2048]
"""

import os
import sys

import numpy as np

sys.path.insert(0, "/opt/trn_rl_repo")

import concourse.bacc as bacc
import concourse.tile as tile
from concourse import mybir
from concourse.masks import make_identity

F32 = mybir.dt.float32
U8 = mybir.dt.uint8
BF16 = mybir.dt.bfloat16
FP8 = mybir.dt.float8e4

import ml_dtypes

MM_DT = BF16
NP_MM_DT = ml_dtypes.bfloat16
NP_FP8 = ml_dtypes.float8_e4m3

TOPK_DT = BF16 if os.environ.get("CLIP_TOPK_DT", "bf16") == "bf16" else F32

B = 128
N = 2048
F_IMG = 1024
F_TXT = 512
D = 512
K = 127          # topK = B - 1
NCORES = 8
BLOC = B // NCORES   # 16 rows per core
NEG = -1e30

KC = D // 128        # 4 contraction chunks of 128
NCH = N // 128       # 16 row-chunks of 128
NGROUP = 2
GR = BLOC // NGROUP  # 8 rows per epilogue group


def declare_params(nc):
    p = {}
    p["imgT"] = nc.declare_dram_parameter("imgT", [F_IMG, BLOC], MM_DT, isOutput=False)
    p["txtT"] = nc.declare_dram_parameter("txtT", [F_TXT, BLOC], MM_DT, isOutput=False)
    p["othT8"] = nc.declare_dram_parameter("othT8", [BLOC, F_TXT, N], FP8, isOutput=False)
    p["othTr"] = nc.declare_dram_parameter("othTr", [BLOC, F_TXT, N], FP8, isOutput=False)
    p["w_img"] = nc.declare_dram_parameter("w_img", [F_IMG, D], MM_DT, isOutput=False)
    p["w_txt"] = nc.declare_dram_parameter("w_txt", [F_TXT, D], MM_DT, isOutput=False)
    p["w_txt8"] = nc.declare_dram_parameter("w_txt8", [F_TXT, D], FP8, isOutput=False)
    p["w_txtT"] = nc.declare_dram_parameter("w_txtT", [D, F_TXT], MM_DT, isOutput=False)
    p["m_lt"] = nc.declare_dram_parameter("m_lt", [BLOC, K + 1], U8, isOutput=False)
    p["m_eq"] = nc.declare_dram_parameter("m_eq", [BLOC, K + 1], U8, isOutput=False)
    p["ls"] = nc.declare_dram_parameter("ls", [1, 1], F32, isOutput=False)
    p["out"] = nc.declare_dram_parameter("out", [BLOC, K + 1], F32, isOutput=True)
    return p


def _build_kernel(tc, p, rep=""):
    nc = tc.nc
    out_dram = p["out"]

    Act = mybir.ActivationFunctionType

    # Pre-load the one activation table that serves every func we use
    # (square/ln/exp/copy). Without this, the table-load pass picks
    # first-match sets per function and the prologue's Ln->Exp pairs
    # cause 6 x 1.28us reload churn on the in-order Act queue.
    from concourse.hw_specs import get_activation_tables
    _tables = get_activation_tables(nc.m.arch)
    _need = {Act.Square, Act.Ln, Act.Exp, Act.Copy}
    _set_id = next(i for i, (_n, _f) in enumerate(_tables.items()) if _need <= _f)
    nc.scalar.add_instruction(mybir.InstLoadActFuncSet(
        name=nc.get_next_instruction_name(), ins=[], outs=[],
        act_func_set_id=_set_id))

    with (
        tc.tile_pool(name=f"{rep}weights", bufs=1) as wpool,
        tc.tile_pool(name=f"{rep}small", bufs=1) as small,
        tc.tile_pool(name=f"{rep}xt", bufs=2) as xt_pool,
        tc.tile_pool(name=f"{rep}ps_e", bufs=4, space="PSUM") as ps_e,
        tc.tile_pool(name=f"{rep}ps_num", bufs=2, space="PSUM") as ps_num,
        tc.tile_pool(name=f"{rep}ps_tp", bufs=1, space="PSUM") as ps_tp,
    ):
        # ---------------- prologue: weights + embeddings ----------------
        prologue_psum = tc.tile_pool(name=f"{rep}ps_misc", bufs=1, space="PSUM")
        ps_misc = prologue_psum.__enter__()
        w_img_sb = wpool.tile([128, F_IMG // 128, D], MM_DT)
        nc.sync.dma_start(w_img_sb, p["w_img"][:].rearrange("(k p) d -> p k d", p=128))
        w_txt_sb = wpool.tile([128, KC, D], MM_DT)
        nc.sync.dma_start(w_txt_sb, p["w_txt"][:].rearrange("(k p) d -> p k d", p=128))
        w_txtT_sb = wpool.tile([128, KC, F_TXT], MM_DT)
        nc.sync.dma_start(w_txtT_sb, p["w_txtT"][:].rearrange("(k p) d -> p k d", p=128))
        w8_sb = wpool.tile([128, KC, D], FP8)
        nc.sync.dma_start(w8_sb, p["w_txt8"][:].rearrange("(k p) d -> p k d", p=128))

        imgT_sb = small.tile([128, F_IMG // 128, BLOC], MM_DT)
        nc.sync.dma_start(imgT_sb, p["imgT"][:].rearrange("(k p) m -> p k m", p=128))
        txtT_sb = small.tile([128, KC, BLOC], MM_DT)
        nc.sync.dma_start(txtT_sb, p["txtT"][:].rearrange("(k p) m -> p k m", p=128))

        m_lt_sb = small.tile([BLOC, K + 1], U8)
        nc.sync.dma_start(m_lt_sb, p["m_lt"][:])
        m_eq_sb = small.tile([BLOC, K + 1], U8)
        nc.sync.dma_start(m_eq_sb, p["m_eq"][:])

        identity = small.tile([128, 128], F32)
        make_identity(nc, identity)

        # img = imgT.T @ W_img   -> [16, 512] (accumulate 8 k-chunks)
        img_ps = ps_misc.tile([BLOC, D], F32, tag="misc")
        nkc_img = F_IMG // 128
        for k in range(nkc_img):
            nc.tensor.matmul(
                img_ps,
                lhsT=imgT_sb[:, k, :],
                rhs=w_img_sb[:, k, :],
                start=(k == 0),
                stop=(k == nkc_img - 1),
            )
        txt_ps = ps_misc.tile([BLOC, D], F32, tag="misc")
        for k in range(KC):
            nc.tensor.matmul(
                txt_ps,
                lhsT=txtT_sb[:, k, :],
                rhs=w_txt_sb[:, k, :],
                start=(k == 0),
                stop=(k == KC - 1),
            )

        # normalize rows of img / txt (copy PSUM->SBUF first: DVE reads
        # at most one PSUM operand)
        img_sb = small.tile([BLOC, D], F32)
        nc.vector.tensor_copy(img_sb, img_ps)
        sq_scr = small.tile([BLOC, D], F32)
        img_nsq = small.tile([BLOC, 1], F32)
        nc.scalar.activation(sq_scr, img_sb, Act.Square, accum_out=img_nsq)
        img_rn = small.tile([BLOC, 1], F32)
        nc.scalar.activation(img_rn, img_nsq, Act.Ln)
        nc.scalar.activation(img_rn, img_rn, Act.Exp, scale=-0.5)
        img_n = small.tile([BLOC, D], F32)
        nc.vector.tensor_scalar_mul(img_n, img_sb, scalar1=img_rn)

        txt_sb = small.tile([BLOC, D], F32)
        nc.vector.tensor_copy(txt_sb, txt_ps)
        sq_scr2 = small.tile([BLOC, D], F32)
        txt_nsq = small.tile([BLOC, 1], F32)
        nc.scalar.activation(sq_scr2, txt_sb, Act.Square, accum_out=txt_nsq)
        txt_rn = small.tile([BLOC, 1], F32)
        nc.scalar.activation(txt_rn, txt_nsq, Act.Ln)
        nc.scalar.activation(txt_rn, txt_rn, Act.Exp, scale=-0.5)
        txt_n = small.tile([BLOC, D], F32)
        nc.vector.tensor_scalar_mul(txt_n, txt_sb, scalar1=txt_rn)

        # logit_in (unscaled) = rowsum(img_n * txt_n)
        prod_it = small.tile([BLOC, D], F32)
        nc.vector.tensor_mul(prod_it, img_n, txt_n)
        sq_scr3 = small.tile([BLOC, D], F32)
        li_raw = small.tile([BLOC, 1], F32)
        nc.scalar.activation(sq_scr3, prod_it, Act.Copy, accum_out=li_raw)

        # broadcast ls to [128,1] via DMA (src partition stride 0)
        import concourse.bass as bass_mod
        ls_ap = p["ls"][:]
        ls_b128 = bass_mod.AP(
            tensor=ls_ap.tensor, offset=ls_ap.offset, ap=[[0, 128], [1, 1]]
        )
        ls128 = small.tile([128, 1], F32)
        nc.sync.dma_start(ls128, ls_b128)
        sc16 = small.tile([BLOC, 1], F32)
        nc.scalar.activation(sc16, ls128[0:BLOC, :], Act.Exp)

        li = small.tile([BLOC, 1], F32)
        nc.vector.tensor_mul(li, li_raw, sc16)

        # img_n^T  [512, 16] via PE transposes of [16,128] slices
        imgnT_sb = small.tile([128, KC, BLOC], MM_DT)
        for c in range(KC):
            tp_ps = ps_misc.tile([128, BLOC], F32, tag="misc")
            nc.tensor.transpose(tp_ps, img_n[:, 128 * c:128 * (c + 1)],
                                identity[:BLOC, :BLOC])
            nc.vector.tensor_copy(imgnT_sb[:, c, :], tp_ps)

        # V[k, b] = sum_j W_txt[k, j] img_n[b, j]  -> v_sb [128, KC, 16]
        v_sb = small.tile([128, KC, BLOC], MM_DT)
        for kcc in range(KC):
            v_ps = ps_misc.tile([128, BLOC], F32, tag="misc")
            for j in range(KC):
                nc.tensor.matmul(
                    v_ps,
                    lhsT=w_txtT_sb[:, j, 128 * kcc:128 * (kcc + 1)],
                    rhs=imgnT_sb[:, j, :],
                    start=(j == 0),
                    stop=(j == KC - 1),
                )
            nc.vector.tensor_copy(v_sb[:, kcc, :], v_ps)
        prologue_psum.__exit__(None, None, None)

        # ---------------- streaming loop over the 16 batch rows ----------------
        nsq_cols = small.tile([128, BLOC * NCH], F32)   # col c = b*16 + nch
        num_cols = small.tile([128, BLOC * NCH], F32)
        rs_cols = small.tile([128, BLOC * NCH], F32)
        log_cols = small.tile([128, BLOC * NCH], F32)
        logT = [small.tile([128, 128], F32, name=f"{rep}logT{t}") for t in range(2)]
        # quarters layout: partition q = b*4 + nq holds row b's quarter nq
        rowlay4 = small.tile([4 * BLOC, N // 4], F32)
        st1 = small.tile([4 * BLOC, 128], F32)          # per-quarter sorted 128
        merge_in = small.tile([BLOC, 512], F32)
        topk_sb = small.tile([BLOC, 128], F32)

        scr_pool = tc.tile_pool(name=f"{rep}scr", bufs=4)
        scr = scr_pool.__enter__()

        def load_xt(b):
            x8 = xt_pool.tile([128, KC, N], FP8, tag="x8", name=f"{rep}x8_{b}")
            nc.sync.dma_start(x8, p["othT8"][b].rearrange("(k p) n -> p k n", p=128))
            xr = xt_pool.tile([128, KC, N], FP8, tag="xr", name=f"{rep}xr_{b}")
            nc.sync.dma_start(xr, p["othTr"][b].rearrange("(k p) n -> p k n", p=128))
            return (x8, xr)

        # hoist xt_0 DMA ahead of the prologue compute
        xts = {0: load_xt(0)}

        # squares: interleaved Act / (DVE copy + Pool square + DVE reduce).
        # GPSIMD cannot read PSUM (walrus birverifier rule), so the Pool
        # path goes through a bf16 SBUF copy of the E tile first.
        SQ_ACT = {0, 2, 3, 5, 6, 8, 9, 11, 12, 14}

        for b in range(BLOC):
            x8, xr = xts.pop(b)
            if b + 1 < BLOC:
                xts[b + 1] = load_xt(b + 1)

            nmp = ps_num.tile([128, BLOC], F32, tag="nm", name=f"{rep}nm_{b}")
            pending = []  # software-pipelined DVE reduces (lag 2) so the
                          # in-order DVE queue never stalls on Pool's mult
            for nch in range(NCH):
                sl = slice(128 * nch, 128 * (nch + 1))
                e_ps = ps_e.tile([128, D], F32, tag="e")
                for kp in range(KC // 2):
                    nc.tensor.matmul(
                        e_ps, lhsT=x8[:, 2 * kp:2 * kp + 2, sl],
                        rhs=w8_sb[:, 2 * kp:2 * kp + 2, :],
                        start=(kp == 0), stop=(kp == KC // 2 - 1),
                        perf_mode=mybir.MatmulPerfMode.DoubleRow,
                    )
                # numerator: num = (x8 + xr) . v   (v stays bf16)
                for kcc in range(KC):
                    nc.tensor.matmul(
                        nmp[:, nch:nch + 1], lhsT=x8[:, kcc, sl],
                        rhs=v_sb[:, kcc, b:b + 1],
                        start=(kcc == 0), stop=False,
                    )
                for kcc in range(KC):
                    nc.tensor.matmul(
                        nmp[:, nch:nch + 1], lhsT=xr[:, kcc, sl],
                        rhs=v_sb[:, kcc, b:b + 1],
                        start=False, stop=(kcc == KC - 1),
                    )
                col = nsq_cols[:, BLOC * b + nch: BLOC * b + nch + 1]
                if nch in SQ_ACT:
                    nc.scalar.activation(e_ps, e_ps, Act.Square, accum_out=col)
                else:
                    e_sb = scr.tile([128, D], BF16, tag="esb")
                    nc.vector.tensor_copy(e_sb, e_ps)
                    s = scr.tile([128, D], BF16, tag="sp")
                    nc.gpsimd.tensor_mul(s, e_sb, e_sb)
                    pending.append((col, s))
                    if len(pending) > 2:
                        pcol, ps_ = pending.pop(0)
                        nc.vector.tensor_reduce(out=pcol, in_=ps_,
                                                op=mybir.AluOpType.add,
                                                axis=mybir.AxisListType.X)
            for pcol, ps_ in pending:
                nc.vector.tensor_reduce(out=pcol, in_=ps_,
                                        op=mybir.AluOpType.add,
                                        axis=mybir.AxisListType.X)
            nc.vector.tensor_copy(num_cols[:, BLOC * b:BLOC * (b + 1)], nmp)
        scr_pool.__exit__(None, None, None)

        # ---------------- epilogue ----------------
        # rs = exp(ls - 0.5 ln nsq), logits = num * rs, all in column layout
        nc.scalar.activation(rs_cols, nsq_cols, Act.Ln)
        nc.scalar.activation(rs_cols, rs_cols, Act.Exp, scale=-0.5, bias=ls128)
        nc.vector.tensor_mul(log_cols, num_cols, rs_cols)

        # transpose to [c = b*16+nch, p] and scatter quarters to partitions:
        # rowlay4[c//4, (c%4)*128 + p] = logT[c, p]  (c//4 = b*4 + nch//4)
        for t in range(2):
            tp = ps_tp.tile([128, 128], F32, tag="tp")
            nc.tensor.transpose(tp, log_cols[:, 128 * t:128 * (t + 1)], identity)
            nc.vector.tensor_copy(logT[t], tp)
            nc.sync.dma_start(
                rowlay4[32 * t:32 * (t + 1), :].rearrange(
                    "q (c p) -> q c p", c=4),
                logT[t][:],
            )

        # stage 1: sorted top-128 of each quarter  [64, 512] -> [64, 128]
        for i in range(16):
            nc.vector.max(out=st1[:, 8 * i:8 * i + 8], in_=rowlay4)
            nc.vector.match_replace(
                out=rowlay4, in_to_replace=st1[:, 8 * i:8 * i + 8],
                in_values=rowlay4, imm_value=NEG,
            )

        # concat the 4 sorted lists of each row: merge_in[b, nq*128+k]
        nc.sync.dma_start(
            merge_in[:].rearrange("b (nq k) -> b nq k", nq=4), st1[:]
        )

        # stage 2: top-128 of the 512 candidates per row
        for i in range(16):
            nc.vector.max(out=topk_sb[:, 8 * i:8 * i + 8], in_=merge_in)
            nc.vector.match_replace(
                out=merge_in, in_to_replace=topk_sb[:, 8 * i:8 * i + 8],
                in_values=merge_in, imm_value=NEG,
            )

        # insert logit_in at column i (global row index): masks from host
        shifted = small.tile([BLOC, K + 1], F32)
        nc.vector.tensor_copy(shifted[:, 1:K + 1], topk_sb[:, 0:K])
        nc.vector.tensor_copy(shifted[:, 0:1], topk_sb[:, 0:1])
        outt = small.tile([BLOC, K + 1], F32)
        nc.vector.select(outt, m_lt_sb, on_true=topk_sb, on_false=shifted)
        nc.vector.copy_predicated(outt, m_eq_sb, li.to_broadcast([BLOC, K + 1]))
        nc.sync.dma_start(out_dram[:], outt)

    return out_dram


def build_module(reps=1):
    nc = bacc.Bacc("TRN2", target_bir_lowering=False, debug=False, num_devices=NCORES)
    with tile.TileContext(nc) as tc:
        p = declare_params(nc)
        for r in range(reps):
            _build_kernel(tc, p, rep=f"r{r}_" if reps > 1 else "")
    nc.compile()
    return nc


def make_in_maps(input_images, input_texts, other_texts, W_img, W_txt, logit_scale):
    input_images = np.asarray(input_images, np.float32)
    input_texts = np.asarray(input_texts, np.float32)
    other_texts = np.asarray(other_texts, np.float32)
    W_img = np.ascontiguousarray(np.asarray(W_img, np.float32))
    W_txt = np.ascontiguousarray(np.asarray(W_txt, np.float32))
    W_txtT = np.ascontiguousarray(W_txt.T)
    ls = np.float32(np.asarray(logit_scale).reshape(-1)[0])

    cols = np.arange(K + 1)
    in_maps = []
    for c in range(NCORES):
        r = slice(BLOC * c, BLOC * (c + 1))
        gi = np.arange(BLOC * c, BLOC * (c + 1))[:, None]  # global row ids
        othT = np.ascontiguousarray(other_texts[r].transpose(0, 2, 1))
        oth8 = othT.astype(NP_FP8)
        othr = (othT - oth8.astype(np.float32)).astype(NP_FP8)
        in_maps.append({
            "imgT": np.ascontiguousarray(input_images[r].T).astype(NP_MM_DT),
            "txtT": np.ascontiguousarray(input_texts[r].T).astype(NP_MM_DT),
            "othT8": oth8,
            "othTr": othr,
            "w_img": W_img.astype(NP_MM_DT),
            "w_txt": W_txt.astype(NP_MM_DT),
            "w_txt8": W_txt.astype(NP_FP8),
            "w_txtT": W_txtT.astype(NP_MM_DT),
            "m_lt": (cols[None, :] < gi).astype(np.uint8),
            "m_eq": (cols[None, :] == gi).astype(np.uint8),
            "ls": np.array([[ls]], np.float32),
        })
    return in_maps


_NC_CACHE = {}


def kernel(input_images, input_texts, other_texts, W_img, W_txt, logit_scale):
    from concourse.bass_utils import run_bass_kernel_spmd

    if "nc" not in _NC_CACHE:
        _NC_CACHE["nc"] = build_module()
    nc = _NC_CACHE["nc"]

    in_maps = make_in_maps(
        input_images, input_texts, other_texts, W_img, W_txt, logit_scale
    )
    res = run_bass_kernel_spmd(nc, in_maps, list(range(NCORES)))
    _NC_CACHE["last_result"] = res
    return np.concatenate([res.results[c]["out"] for c in range(NCORES)], axis=0)
